# revision 5
# baseline (speedup 1.0000x reference)
"""Trainium2 Bass kernel: gamma-scaled negative squared-distance matrix.

Computes out[b,k] = -gamma[k] * (||D[b]||^2 + ||W[k]||^2 - 2*D[b].W[k])
for D [16384,512], W [1000,512], gamma [1000] -> out [16384,1000] fp32.

Strategy (k-major fp8 DoubleRow)
--------------------------------
Data-parallel over 8 NeuronCores: D sharded along batch (2048 rows/core),
weights replicated. Per core the device computes

    X'[k, b] = (2*D[b].W[k] - wsq[k] + C) / S

k-major: psum partition = k (128-row kt block), free = batch. The host
finishes with the affine  out[b,k] = gamma[k] * (S*X'[k,b] - C - dsq[b])
(same class of host prep/post as the original baseline's 2*gamma*W fold
and d_sq/w_sq precomputes). C=512 centers X' so the fp8e3 (e3m4) output
stripe costs ~1e-3 rel err; S=32 keeps it in e3m4 range.

Matmuls run in fp8e4 (e4m3, TRN max +-240) with perf_mode=DoubleRow:
operands are 3D APs [128, 2, N] packing two 128-deep contraction chunks
per instruction. Weights are stationary (4x fewer LDWEIGHTS than
batch-stationary). Measured DR issue rate is 512 cycles per [128,512]
matmul at 2.4 GHz = the 157 TF/s hw floor: mains = 64 mm = 13.8 us.

Work is ordered as 16 units of 4 matmuls: unit u = (kt, h) covering
batch-half h (columns h*1024..), all h=0 units first. Each unit is one
psum accumulation group pair on banks (u%4)*2, (u%4)*2+1 (4-deep
rotation), so the PE only ever waits for the epilogue of unit u-4.
Epilogue: one fused pass per unit, alternating engines by unit parity
(ACT: Identity(psum*1/S + bias[k]); DVE: (psum*1/S) + scalar2[k]),
written straight to the fp8 staging stripe; gpsimd cannot access PSUM
on TRN2. Stores are per-unit [128,1024] halves on the sync queue.

Scheduling: everything startable is emitted PRE-BLOCK (load DMA issues,
warm-up matmuls, act-table prime, memset) so it runs ~1.3 us before the
Block-entry barrier. dt is laid out [128, (bh,c), 1024] so each 256KB
quarter feeds a specific unit range: sync queue carries the two bh0
pieces (gate unit 0), scalar queue wt halves + bh1a, gpsimd/SWDGE gets
only the last-needed piece (bh1b) since SWDGE descriptor generation
adds ~3.4 us latency. NWARM DoubleRow warm-ups on a zeroed scratch tile
bridge the load window so the HAM clock (0.65->1.2->2.4 GHz after ~3 us
sustained) is at full rate when mains start, and the PE stream never
gaps >0.5 us (which triggers a ~50%-duty rethrottle).
"""

import os
import sys
import types
from contextlib import ExitStack

sys.path.insert(0, "/opt/trn_rl_repo")

import numpy as np
import ml_dtypes


def _install_ntff_hook():
    """The agent image's ``antenv`` lacks ``axon_hooks``; synthesize it and
    register the ctypes NTFF profile hook so trace=True works."""
    try:
        import antenv.axon_hooks  # noqa: F401

        return
    except ImportError:
        pass
    try:
        import antenv

        mod = types.ModuleType("antenv.axon_hooks")
        mod._hook = None
        mod.set_axon_ntff_profile_hook = lambda h: setattr(mod, "_hook", h)
        mod.get_axon_ntff_profile_hook = lambda: mod._hook
        sys.modules["antenv.axon_hooks"] = mod
        antenv.axon_hooks = mod
        so = "/opt/axon/libaxon_pjrt.so"
        if os.path.exists(so):
            from trn_agent_boot.trn_boot import _ntff_profile_via_ctypes

            mod._hook = _ntff_profile_via_ctypes(so)
    except Exception:
        pass


_install_ntff_hook()

import concourse.bass as bass  # noqa: E402,F401
from concourse import bacc, mybir  # noqa: E402
from concourse import bass_utils  # noqa: E402

B, F, K = 16384, 512, 1000
NCORES = 8
BS = B // NCORES          # 2048 batch rows per core
P = 128
KP = 1024                 # k padded to 8*128
NKT = KP // P             # 8 kt blocks
BC = 512                  # psum-bank batch chunk
HB = 1024                 # batch-half (2 chunks) per unit
NOT = 4                   # output staging stripes
NWARM = 10                # DR warm-up matmuls bridging the input window

OUT_DT = os.environ.get("KV2_OUT", "f8e3")
C_CENTER = 512.0
S_SCALE = 32.0 if OUT_DT == "f8e3" else 1.0

_NC_CACHE = None

# unit u -> (kt, h): all h=0 units first, then h=1
UNITS = [(u % NKT, u // NKT) for u in range(2 * NKT)]


def _unit_sem_count(u):
    """(engine, count) identifying unit u's epilogue completion: ACT owns
    even units, DVE odd units."""
    if u % 2 == 0:
        return "a", u // 2 + 1
    return "d", (u + 1) // 2


def _build_nc():
    nc = bacc.Bacc("TRN2", target_bir_lowering=False, debug=False)
    f8 = mybir.dt.float8e4
    f32 = mybir.dt.float32
    odt = mybir.dt.float8e3 if OUT_DT == "f8e3" else mybir.dt.float16
    Identity = mybir.ActivationFunctionType.Identity
    Alu = mybir.AluOpType
    DR = mybir.MatmulPerfMode.DoubleRow

    # dt: [128, (bh*4 + c), 1024]: dt[p, bh*4+c, bl] = D[bh*1024+bl, c*128+p]
    dt = nc.dram_tensor("dt", [P, 8, HB], f8, kind="ExternalInput").ap()
    wt = nc.dram_tensor("wt", [P, 4 * NKT, P], f8, kind="ExternalInput").ap()
    cols = nc.dram_tensor("cols", [P, NKT], f32, kind="ExternalInput").ap()
    o = nc.dram_tensor("o", [KP, BS], odt, kind="ExternalOutput").ap()

    with ExitStack() as ctx:
        dt_sb = ctx.enter_context(nc.sbuf_tensor("dt_sb", [P, 8, HB], f8)).ap()
        wt_sb = ctx.enter_context(nc.sbuf_tensor("wt_sb", [P, 4 * NKT, P], f8)).ap()
        cols_sb = ctx.enter_context(nc.sbuf_tensor("cols_sb", [P, NKT], f32)).ap()
        warm_sb = ctx.enter_context(nc.sbuf_tensor("warm_sb", [P, 2, BC], f8)).ap()
        scr_sb = ctx.enter_context(nc.sbuf_tensor("scr_sb", [P, 1], f32)).ap()
        ots = [
            ctx.enter_context(nc.sbuf_tensor(f"ot{i}", [P, BS], odt)).ap()
            for i in range(NOT)
        ]
        banks = ctx.enter_context(nc.psum_tensor("banks", [P, 8 * BC], f32)).ap()

        s_ws = ctx.enter_context(nc.semaphore("s_ws"))
        s_wtaa = ctx.enter_context(nc.semaphore("s_wtaa"))
        s_wtab = ctx.enter_context(nc.semaphore("s_wtab"))
        s_wtb = ctx.enter_context(nc.semaphore("s_wtb"))
        s_q = [ctx.enter_context(nc.semaphore(f"s_q{i}")) for i in range(4)]
        s_cols = ctx.enter_context(nc.semaphore("s_cols"))
        s_mm = ctx.enter_context(nc.semaphore("s_mm"))
        s_xa = ctx.enter_context(nc.semaphore("s_xa"))
        s_xd = ctx.enter_context(nc.semaphore("s_xd"))
        s_st = [
            [ctx.enter_context(nc.semaphore(f"s_st{i}_{h}")) for h in range(2)]
            for i in range(NOT)
        ]

        def sems_of(tag):
            return s_xa if tag == "a" else s_xd

        def wtb_ap(kt, j):
            c0 = kt * 4 + 2 * j
            return wt_sb[:, c0 : c0 + 2, :]

        def dtb_ap(h, j, bi):
            c0 = h * 4 + 2 * j
            return dt_sb[:, c0 : c0 + 2, bi * BC : (bi + 1) * BC]

        def unit_banks(u):
            off = (u % 4) * 2 * BC
            return banks[:, off : off + 2 * BC]

        def unit_bank(u, bi):
            off = ((u % 4) * 2 + bi) * BC
            return banks[:, off : off + BC]

        # ---- pre-block: load issues, table prime, memset, warm-ups ----
        # u0 needs bh0 (sync queue) + wtA (scalar queue); aggregate load BW
        # is HBM-capped (~330 GB/s/core), so the late-needed bh1 goes on the
        # high-latency SWDGE queue and wtB/cols trail on the scalar queue.
        nc.sync.dma_start(dt_sb[:, 0:2, :], dt[:, 0:2, :]).then_inc(s_q[0], 16)
        nc.sync.dma_start(wt_sb[:, 0:8, :], wt[:, 0:8, :]).then_inc(
            s_wtaa, 16
        )
        nc.sync.dma_start(wt_sb[:, 8:16, :], wt[:, 8:16, :]).then_inc(
            s_wtab, 16
        )
        nc.scalar.dma_start(dt_sb[:, 2:4, :], dt[:, 2:4, :]).then_inc(
            s_q[1], 16
        )
        nc.scalar.dma_start(wt_sb[:, 16:32, :], wt[:, 16:32, :]).then_inc(
            s_wtb, 16
        )
        nc.scalar.dma_start(cols_sb[:], cols[:]).then_inc(s_cols, 16)
        nc.scalar.activation(
            scr_sb[:, :1],
            nc.const_aps.scalar_like(0.0, scr_sb[:, :1]),
            Identity,
            bias=0.0,
        )
        nc.vector.memset(warm_sb[:].bitcast(mybir.dt.uint32), 0).then_inc(
            s_ws, 1
        )
        nc.tensor.wait_ge(s_ws, 1)
        for w in range(NWARM):
            nc.tensor.matmul(
                banks[:, 6 * BC : 7 * BC],
                warm_sb[:, :, :P],
                warm_sb[:],
                start=True,
                stop=True,
                perf_mode=DR,
            )

        blk = ctx.enter_context(nc.Block())

        # gates: PE waits these sems before the given unit index
        GATES = {
            0: [(s_wtaa, 16), (s_q[0], 16), (s_q[1], 16)],
            2: [(s_wtab, 16)],
            4: [(s_wtb, 16)],
            8: [(s_q[2], 16)],
        }

        @blk.sync
        def _(sync):
            for u, (kt, h) in enumerate(UNITS):
                if u == 2 * NKT - 1:
                    # last unit: ACT did cols [h*HB, h*HB+BC), DVE the rest.
                    # Scalar stores its own (drain-ordered) piece; store the
                    # DVE piece here so both final stores issue in parallel.
                    sync.wait_ge(s_xd, NKT)
                    sync.dma_start(
                        o[kt * P : (kt + 1) * P, h * HB + BC : (h + 1) * HB],
                        ots[kt % NOT][:, h * HB + BC : (h + 1) * HB],
                    ).then_inc(s_st[kt % NOT][h], 16)
                    continue
                tag, cnt = _unit_sem_count(u)
                sync.wait_ge(sems_of(tag), cnt)
                sync.dma_start(
                    o[kt * P : (kt + 1) * P, h * HB : (h + 1) * HB],
                    ots[kt % NOT][:, h * HB : (h + 1) * HB],
                ).then_inc(s_st[kt % NOT][h], 16)

        @blk.gpsimd
        def _(gpsimd):
            # defer the late-needed bh1 load until the critical bh0 piece is
            # off the wire: SWDGE otherwise steals HBM bandwidth from the
            # mains-gating loads (aggregate is capped ~330 GB/s/core)
            gpsimd.wait_ge(s_q[0], 16)
            gpsimd.dma_start(dt_sb[:, 4:8, :], dt[:, 4:8, :]).then_inc(
                s_q[2], 16
            )

        @blk.scalar
        def _(scalar):
            scalar.wait_ge(s_cols, 16)
            for u in range(0, 2 * NKT, 2):
                kt, h = UNITS[u]
                scalar.wait_ge(s_mm, u + 1)
                if kt >= NOT:
                    scalar.wait_ge(s_st[kt % NOT][h], 16 * (kt // NOT))
                nc.scalar.activation(
                    ots[kt % NOT][:, h * HB : (h + 1) * HB],
                    unit_banks(u),
                    Identity,
                    bias=cols_sb[:, kt : kt + 1],
                    scale=1.0 / S_SCALE,
                )
                scalar.drain().then_inc(s_xa, 1)
            # last unit, first bank piece (the DVE handles the second)
            uL = 2 * NKT - 1
            ktL, hL = UNITS[uL]
            scalar.wait_ge(s_mm, uL + 1)
            if ktL >= NOT:
                scalar.wait_ge(s_st[ktL % NOT][hL], 16 * (ktL // NOT))
            nc.scalar.activation(
                ots[ktL % NOT][:, hL * HB : hL * HB + BC],
                unit_bank(uL, 0),
                Identity,
                bias=cols_sb[:, ktL : ktL + 1],
                scale=1.0 / S_SCALE,
            )
            scalar.drain().then_inc(s_xa, 1)
            scalar.dma_start(
                o[ktL * P : (ktL + 1) * P, hL * HB : hL * HB + BC],
                ots[ktL % NOT][:, hL * HB : hL * HB + BC],
            ).then_inc(s_st[ktL % NOT][hL], 16)

        @blk.vector
        def _(vector):
            vector.wait_ge(s_cols, 16)
            for u in range(1, 2 * NKT, 2):
                kt, h = UNITS[u]
                vector.wait_ge(s_mm, u + 1)
                if kt >= NOT:
                    vector.wait_ge(s_st[kt % NOT][h], 16 * (kt // NOT))
                last = u == 2 * NKT - 1
                nc.vector.tensor_scalar(
                    ots[kt % NOT][
                        :, h * HB + (BC if last else 0) : (h + 1) * HB
                    ],
                    unit_bank(u, 1) if last else unit_banks(u),
                    1.0 / S_SCALE,
                    cols_sb[:, kt : kt + 1],
                    Alu.mult,
                    Alu.add,
                ).then_inc(s_xd, 1)

        @blk.tensor
        def _(tensor):
            def reuse_wait(u):
                if u < 4:
                    return None
                tag, cnt = _unit_sem_count(u - 4)
                return sems_of(tag), cnt

            for u, (kt, h) in enumerate(UNITS):
                for sem, val in GATES.get(u, ()):
                    tensor.wait_ge(sem, val)
                if u == 0 and reuse_wait(0):
                    sem, val = reuse_wait(0)
                    tensor.wait_ge(sem, val)
                for j in range(2):
                    for bi in range(2):
                        if j == 1 and bi == 1:
                            # hoist the next unit's bank-reuse wait here so
                            # the PE stream doesn't restart cold at the
                            # unit boundary
                            if u + 1 < len(UNITS) and u + 1 not in GATES:
                                nxt = reuse_wait(u + 1)
                                if nxt:
                                    tensor.wait_ge(nxt[0], nxt[1])
                        mmi = nc.tensor.matmul(
                            unit_bank(u, bi),
                            wtb_ap(kt, j),
                            dtb_ap(h, j, bi),
                            start=(j == 0),
                            stop=(j == 1),
                            perf_mode=DR,
                        )
                mmi.then_inc(s_mm, 1)
                # units with explicit gates do their reuse wait at the top
                if u + 1 in GATES:
                    nxt = reuse_wait(u + 1)
                    if nxt:
                        tensor.wait_ge(nxt[0], nxt[1])

    nc.compile()
    return nc


def _get_nc():
    global _NC_CACHE
    if _NC_CACHE is None:
        _NC_CACHE = _build_nc()
    return _NC_CACHE


def _prep(D, weight, gamma):
    D = np.asarray(D, dtype=np.float32)
    weight = np.asarray(weight, dtype=np.float32)
    gamma = np.asarray(gamma, dtype=np.float32)
    f8 = ml_dtypes.float8_e4m3

    # dt image [128, 8, 1024]: dt[p, bh*4+c, bl] = D[bh*1024+bl, c*128+p]
    # D.T [512, B] -> [4c, 128p, nbh, 1024bl] -> [p, bh, c, bl]
    nbh = B // HB
    DT = (
        np.clip(D.T, -240, 240)
        .reshape(4, P, nbh, HB)
        .transpose(1, 2, 0, 3)
        .astype(f8)
    )  # [128, nbh, 4, 1024]

    # wt image [128, 32, 128]: wt[p, kt*4 + c, kl] = 2*W[kt*128+kl, c*128+p]
    W2 = np.zeros((KP, F), np.float32)
    W2[:K] = 2.0 * weight
    WT = np.ascontiguousarray(
        np.clip(W2, -240, 240)
        .reshape(NKT, P, 4, P)
        .transpose(3, 0, 2, 1)
        .reshape(P, 4 * NKT, P)
    ).astype(f8)

    w_sq = np.zeros(KP, np.float64)
    w_sq[:K] = np.square(weight, dtype=np.float64).sum(axis=1)
    COLS = np.ascontiguousarray(
        ((C_CENTER - w_sq) / S_SCALE).reshape(NKT, P).T
    ).astype(np.float32)

    d_sq = np.square(D, dtype=np.float64).sum(axis=1).astype(np.float32)

    bh_per_core = BS // HB  # 2
    in_maps = []
    for ci in range(NCORES):
        dtc = DT[:, ci * bh_per_core : (ci + 1) * bh_per_core]  # [128,2,4,1024]
        in_maps.append(
            {
                "dt": np.ascontiguousarray(dtc.reshape(P, 8, HB)),
                "wt": WT,
                "cols": COLS,
            }
        )
    return in_maps, d_sq, gamma


def kernel_with_results(D, weight, gamma, trace=False):
    nc = _get_nc()
    in_maps, d_sq, gamma = _prep(D, weight, gamma)
    res = bass_utils.run_bass_kernel_spmd(
        nc, in_maps, core_ids=list(range(NCORES)), trace=trace
    )
    X = np.concatenate([r["o"] for r in res.results], axis=1)  # [KP, B]
    out = (
        X[:K].T.astype(np.float32) * S_SCALE - C_CENTER - d_sq[:, None]
    ) * gamma[None, :]
    return out, res


def kernel(D, weight, gamma):
    out, _ = kernel_with_results(D, weight, gamma)
    return out
